# revision 19
# baseline (speedup 1.0000x reference)
"""CARAFE-downsampling Trainium2 kernel (8-core SPMD, full I/O contract).

Per core (core = 4n + s; batch n, output-row slab h' in [32s, 32s+32)):

  enc logits fused down+enc (9 taps, C_tap = B_tap @ A on host):
      enc[e, hd, wd] = sum_tap C_tap.T @ xk[:, 2hd+dy, 2wd+dx]
      xk = x rows [64s-1, 64s+64) + mask channel, columns pre-deinterleaved
      (even/odd) on host so matmul rhs reads are step-1.
  kw = softmax_k(enc) in [k-partition, (hd, wd)-free] layout: exp on ACT,
      sum-over-k via ones-matmul on PE, reciprocal + normalize on DVE.
      kw -> DRAM scratch (piecewise, per 2-chunk group) -> partition-
      broadcast DMA back as kwb[(q,co), k, hh, w'] tiles (64-way partition
      replication), exports/broadcasts split across sync/scalar queues.
  G duplicated into both partition halves in one matmul (out_w tiled to
      [128, 128] lhsT; PE cost is column-bound so duplication is free):
      pg[(q,co), u] = sum_{c,t} out_w[co, 4c+t] x[c, row, u-2], evicted
      straight to a single shifted-read tile g2[(q,co), r, u'] =
      G[co, r, 128q + u' - 2] (132 cols) - q=0 cols on ACT, q=1 on DVE.
      All 25 tap shifts are then plain AP offset slices (no SWDGE copies).
  products: per half H, 25 taps k=(ki,kj):
      stg[(q,co), hh, w'] = g2[:, 8H+ki:+8, kj:kj+128] * kwb  (DVE 2x /
      Pool split), accumulated with identity matmuls into PSUM on PE;
      out_b added during ACT eviction (bias vector).
"""
import os

import numpy as np
import ml_dtypes

import concourse.bass as bass
import concourse.tile as tile
from concourse import bacc, mybir, masks
from concourse.bass_utils import run_bass_kernel_spmd

F32 = mybir.dt.float32
BF16 = mybir.dt.bfloat16

N_CORES = 8


# ----------------------------------------------------------------------------
# device program
# ----------------------------------------------------------------------------
def build_nc():
    nc = bacc.Bacc(None, target_bir_lowering=False)

    xk_d = nc.dram_tensor("xk", [2, 65, 69, 129], BF16, kind="ExternalInput")
    xb_d = nc.dram_tensor("xb", [2, 128, 20, 264], BF16, kind="ExternalInput")
    ct_d = nc.dram_tensor("ctap", [128, 5, 25], BF16, kind="ExternalInput")
    cm_d = nc.dram_tensor("ctm", [9, 25], BF16, kind="ExternalInput")
    xm_d = nc.dram_tensor("xm", [9, 32, 128], BF16, kind="ExternalInput")
    w4_d = nc.dram_tensor("w4", [2, 128, 128], BF16, kind="ExternalInput")
    ob_d = nc.dram_tensor("obv", [128, 1], F32, kind="ExternalInput")
    kwd_d = nc.dram_tensor("kwd", [2, 2, 8, 25, 128], BF16, kind="Internal")
    # out[H, (q,co), hh, w']; h' = 16H + 2hh + q
    out_d = nc.dram_tensor("out", [2, 128, 8, 128], BF16, kind="ExternalOutput")

    ctx = nc.allow_low_precision(reason="bf16 pipeline; validated ~1% rel err")
    ctx.__enter__()
    with tile.TileContext(nc) as tc:
        with (
            tc.tile_pool(name="consts", bufs=1) as consts,
            tc.tile_pool(name="xkp", bufs=3) as xkp,
            tc.tile_pool(name="xbp", bufs=1) as xbp,
            tc.tile_pool(name="g2p", bufs=1) as g2p,
            tc.tile_pool(name="kwp", bufs=1) as kwp,
            tc.tile_pool(name="encp", bufs=3) as encp,
            tc.tile_pool(name="kwbp", bufs=1) as kwbp,
            tc.tile_pool(name="stgp", bufs=2) as stgp,
            tc.tile_pool(name="resp", bufs=2) as resp,
            tc.tile_pool(name="pse", bufs=3, space="PSUM") as pse,
            tc.tile_pool(name="psg", bufs=2, space="PSUM") as psg,
            tc.tile_pool(name="pss", bufs=1, space="PSUM") as pss,
            tc.tile_pool(name="psacc", bufs=1, space="PSUM") as psacc,
        ):
            # ---- constants ----
            ctap = consts.tile([128, 5, 25], BF16)
            nc.sync.dma_start(ctap[:], ct_d[:])
            ctm = consts.tile([9, 25], BF16)
            nc.sync.dma_start(ctm[:], cm_d[:])
            xm = consts.tile([9, 32, 128], BF16)
            nc.scalar.dma_start(xm[:], xm_d[:])
            w4t = consts.tile([128, 2, 128], BF16)
            nc.scalar.dma_start(w4t[:], w4_d[:].transpose([1, 0, 2]))
            obv = consts.tile([128, 1], F32)
            nc.sync.dma_start(obv[:], ob_d[:])
            identb = consts.tile([128, 128], BF16)
            masks.make_identity(nc, identb[:])
            ones25 = consts.tile([25, 25], BF16)
            nc.gpsimd.memset(ones25[:], 1.0)

            # ---- input streams (xk on scalar queue; sync queue is the
            # kw export/broadcast pipeline) ----
            xkc = [[], []]

            def issue_xk(cp):
                # xkdup tiles [128, 18, 128] (one per 2-chunk pair):
                # partitions 0:64 = channels as-is; 64:128 = same channels
                # shifted (+1 col for par0 pairing dx=0/2, +2 rows for
                # par1 pairing dy=0/2).
                r0 = 16 * cp
                for par in range(2):
                    t = xkp.tile([128, 18, 128], BF16, tag=f"xk{par}",
                                 name=f"xk{par}_{cp}")
                    off = 1 if par == 0 else 2 * 129
                    e = xk_d[par, 0:64, r0:r0 + 18, 0:128]
                    nc.scalar.dma_start(t[0:64], e)
                    nc.scalar.dma_start(
                        t[64:128],
                        bass.AP(e.tensor, e.offset + off,
                                [list(x) for x in e.ap]))
                    xkc[par].append(t)

            for cp in range(3):
                issue_xk(cp)
            xbt = xbp.tile([128, 2, 20, 264], BF16)
            nc.sync.dma_start(xbt[:], xb_d[:].transpose([1, 0, 2, 3]))

            kwn = kwp.tile([25, 32, 128], BF16)
            g2 = g2p.tile([128, 20, 132], BF16)
            kwec = {}

            def enc_chunk(cc):
                pe = pse.tile([25, 4, 128], F32, name=f"pe{cc}", tag="pe")
                cp, rb = cc // 2, 8 * (cc % 2)
                for dy in range(3):  # tap pairs (dy,0)+(dy,2)
                    nc.tensor.matmul(pe[:], ctap[:, dy, :],
                                     xkc[0][cp][:, rb + dy:rb + dy + 8:2, :],
                                     start=(dy == 0), stop=False)
                # pair (0,1)+(2,1), then center (1,1) (zero bottom rows)
                nc.tensor.matmul(pe[:], ctap[:, 3, :],
                                 xkc[1][cp][:, rb:rb + 8:2, :],
                                 start=False, stop=False)
                nc.tensor.matmul(pe[:], ctap[:, 4, :],
                                 xkc[1][cp][:, rb + 1:rb + 9:2, :],
                                 start=False, stop=False)
                # mask/bias taps (9-contraction)
                nc.tensor.matmul(pe[:], ctm[:],
                                 xm[:, 4 * cc:4 * cc + 4, :],
                                 start=False, stop=True)
                kwec[cc] = encp.tile([25, 4, 128], BF16, tag="kwe",
                                    name=f"kwe{cc}")
                nc.scalar.activation(kwec[cc][:], pe[:],
                                     mybir.ActivationFunctionType.Exp)

            def sum_chunk(cc):
                ps = pss.tile([25, 4, 128], F32, name=f"ps{cc}", tag="ps")
                nc.tensor.matmul(ps[:], ones25[:], kwec[cc][:],
                                 start=True, stop=True)
                rcp = encp.tile([25, 4, 128], F32, tag="rcp",
                               name=f"rcp{cc}")
                nc.vector.reciprocal_approx_fast(rcp[:], ps[:])
                nc.vector.tensor_mul(kwn[:, 4 * cc:4 * cc + 4, :],
                                     kwec[cc][:], rcp[:])

            # kw export + broadcast-back, piecewise per 2-chunk group.
            # half H, c2 in {0,1}: kwn rows hd in [16H+8c2, 16H+8c2+8)
            # = hh in [4c2, 4c2+4) for both q.  q=0 traffic on sync
            # queue, q=1 on scalar (same-queue RAW with its export).
            kwb = {}
            for H in range(2):
                for g in range(5):
                    kwb[(H, g)] = kwbp.tile([128, 8, 5, 128], BF16,
                                            name=f"kwb{H}{g}", tag=f"kwb{H}{g}")

            def kw_piece(H, c2):
                hd0 = 16 * H + 8 * c2
                for q in range(2):
                    nc.gpsimd.dma_start(
                        kwd_d[H, q, 4 * c2:4 * c2 + 4, :, :]
                        .transpose([1, 0, 2]),
                        kwn[:, hd0 + q:hd0 + 8:2, :])
                for g in range(5):
                    for q in range(2):
                        eng = nc.gpsimd if q == 1 else nc.sync
                        eng.dma_start(
                            kwb[(H, g)][64 * q:64 * q + 64,
                                        4 * c2:4 * c2 + 4, :, :],
                            kwd_d[H, q, 4 * c2:4 * c2 + 4, 5 * g:5 * g + 5]
                            .unsqueeze(0)
                            .broadcast_to([64, 4, 5, 128]))

            # ---- enc phase: all 8 chunks; sums pipelined one behind so
            # the PE never waits on the ACT exp eviction; kw pieces fire
            # as soon as their 2-chunk group is normalized ----
            enc_chunk(0)
            for cc in range(1, 8):
                if cc == 2:
                    issue_xk(3)
                enc_chunk(cc)
                sum_chunk(cc - 1)
                if cc % 2 == 0:
                    kw_piece((cc - 2) // 4, ((cc - 2) % 4) // 2)
            sum_chunk(7)
            kw_piece(1, 1)

            # ---- G rows: duplicated-G matmul, direct eviction to g2 ----
            for r in range(20):
                pg = psg.tile([128, 264], F32, tag="pg", name=f"pg{r}")
                for ci in range(2):
                    nc.tensor.matmul(pg[:], w4t[:, ci, :], xbt[:, ci, r, :],
                                     start=(ci == 0), stop=(ci == 1))
                nc.scalar.copy(g2[0:64, r, :], pg[0:64, 0:132])
                nc.vector.tensor_copy(g2[64:128, r, :], pg[64:128, 128:260])

            # ---- products ----
            def prod_half(H):
                acc = [psacc.tile([128, 512], F32, name=f"acc{H}{b}",
                                  tag=f"acc{b}") for b in range(2)]
                for ki in range(5):
                    stg = stgp.tile([128, 8, 5, 128], BF16, tag="stg",
                                    name=f"stg{H}_{ki}")
                    base = g2[:, 8 * H + ki:8 * H + ki + 8, 0:128]
                    # overlapped window AP: [p, hh(+132), kj(+1), w(+1)]
                    gsl = bass.AP(base.tensor, base.offset,
                                  [[base.ap[0][0], 128], [132, 8],
                                   [1, 5], [1, 128]])
                    nc.vector.tensor_mul(stg[:], gsl, kwb[(H, ki)][:])
                    for kj in range(5):
                        for b in range(2):
                            nc.tensor.matmul(
                                acc[b][:], identb[:],
                                stg[:, 4 * b:4 * b + 4, kj, :],
                                start=(ki == 0 and kj == 0),
                                stop=(ki == 4 and kj == 4))
                res = resp.tile([128, 8, 128], BF16, tag="res",
                                name=f"res{H}")
                for b in range(2):
                    nc.scalar.activation(
                        res[:, 4 * b:4 * b + 4, :], acc[b][:],
                        mybir.ActivationFunctionType.Identity,
                        bias=obv[:, 0].unsqueeze(-1))
                eng = nc.sync if H == 0 else nc.scalar
                eng.dma_start(out_d[H], res[:])

            prod_half(0)
            prod_half(1)

    nc.compile()
    ctx.__exit__(None, None, None)
    return nc


# ----------------------------------------------------------------------------
# host side
# ----------------------------------------------------------------------------
def _prep_weights(down_w, down_b, enc_w, enc_b, out_w, out_b):
    A = np.zeros((65, 65), np.float32)
    A[0:64, 0:64] = down_w[:, :, 0, 0]
    A[0:64, 64] = down_b
    A[64, 64] = 1.0
    ctap = np.zeros((65, 9, 25), np.float32)
    for dy in range(3):
        for dx in range(3):
            B = np.zeros((25, 65), np.float32)
            B[:, 0:64] = enc_w[:, :, dy, dx]
            if dy == 1 and dx == 1:
                B[:, 64] = enc_b
            ctap[:, 3 * dy + dx, :] = (B @ A).T
    body, ctm = ctap[0:64], ctap[64]          # [64, 9, 25], [9, 25]
    ctp = np.zeros((128, 5, 25), np.float32)
    for dy in range(3):                        # pairs (dy,0)+(dy,2)
        ctp[0:64, dy] = body[:, 3 * dy + 0]
        ctp[64:128, dy] = body[:, 3 * dy + 2]
    ctp[0:64, 3] = body[:, 1]                  # pair (0,1)+(2,1)
    ctp[64:128, 3] = body[:, 7]
    ctp[0:64, 4] = body[:, 4]                  # center (1,1); bottom 0
    w4 = out_w[:, :, 0, 0].T.reshape(2, 128, 64)
    w4 = np.tile(w4, (1, 1, 2)).astype(ml_dtypes.bfloat16)
    obv = np.tile(out_b, 2).reshape(128, 1).astype(np.float32)
    return (ctp.astype(ml_dtypes.bfloat16), ctm.astype(ml_dtypes.bfloat16),
            w4, obv)


def _slice_core(x, n, s):
    xk = np.zeros((65, 69, 258), np.float32)
    h0 = 64 * s - 1
    lo, hi = max(0, -h0), min(65, 256 - h0)
    xk[0:64, lo:hi, 1:257] = x[n, :, h0 + lo:h0 + hi, :]
    xkp = np.zeros((2, 65, 69, 129), np.float32)
    xkp[0] = xk[:, :, 0::2]
    xkp[1] = xk[:, :, 1::2]
    # mask tensor xm[t, hd, wd]: 1 where tap (dy,dx)=(t//3,t%3) lands on
    # a valid (unpadded) input position
    rv = np.zeros(69, bool)
    rv[lo:hi] = True
    cv = np.zeros(258, bool)
    cv[1:257] = True
    xm = np.zeros((9, 32, 128), np.float32)
    hd, wd = np.arange(32), np.arange(128)
    for t in range(9):
        dy, dx = t // 3, t % 3
        xm[t] = (rv[2 * hd + dy][:, None] & cv[2 * wd + dx][None, :])
    xb = np.zeros((2, 128, 20, 264), np.float32)
    xbv = xb.reshape(256, 20, 264)
    for t in range(4):
        g0 = 64 * t + 16 * s - 2
        lo, hi = max(0, -g0), min(20, 256 - g0)
        xbv[np.arange(64) * 4 + t, lo:hi, 2:258] = x[n, :, g0 + lo:g0 + hi, :]
    return (xkp.astype(ml_dtypes.bfloat16), xb.astype(ml_dtypes.bfloat16),
            xm.astype(ml_dtypes.bfloat16))


_NC_CACHE = None
LAST_EXEC_NS = None


def kernel(x, down_w, down_b, enc_w, enc_b, out_w, out_b):
    global _NC_CACHE, LAST_EXEC_NS
    x = np.asarray(x, np.float32)
    ctp, ctm, w4, obv = _prep_weights(
        np.asarray(down_w, np.float32), np.asarray(down_b, np.float32),
        np.asarray(enc_w, np.float32), np.asarray(enc_b, np.float32),
        np.asarray(out_w, np.float32), np.asarray(out_b, np.float32))
    in_maps = []
    for core in range(N_CORES):
        n, s = core // 4, core % 4
        xkp, xb, xm = _slice_core(x, n, s)
        in_maps.append({"xk": xkp, "xb": xb, "ctap": ctp, "ctm": ctm,
                        "xm": xm, "w4": w4, "obv": obv})
    if _NC_CACHE is None:
        _NC_CACHE = build_nc()
    kw = {}
    if os.environ.get("CARAFE_TRACE"):
        kw = dict(trace=True, tmpdir=os.environ.get("CARAFE_TRACE_DIR"))
    res = run_bass_kernel_spmd(_NC_CACHE, in_maps, list(range(N_CORES)), **kw)
    if res.exec_time_ns is not None:
        LAST_EXEC_NS = res.exec_time_ns
    out = np.zeros((2, 64, 128, 128), np.float32)
    for core in range(N_CORES):
        n, s = core // 4, core % 4
        o = res.results[core]["out"].astype(np.float32)  # (H, (q,co), hh, w')
        o = o.reshape(2, 2, 64, 8, 128)                  # (H, q, co, hh, w')
        # h' = 16H + 2hh + q
        o = o.transpose(2, 0, 3, 1, 4).reshape(64, 32, 128)
        out[n, :, 32 * s:32 * s + 32, :] = o
    return out


# revision 20
# speedup vs baseline: 1.0162x; 1.0162x over previous
"""CARAFE-downsampling Trainium2 kernel (8-core SPMD, full I/O contract).

Per core (core = 4n + s; batch n, output-row slab h' in [32s, 32s+32)):

  enc logits fused down+enc (9 taps, C_tap = B_tap @ A on host):
      enc[e, hd, wd] = sum_tap C_tap.T @ xk[:, 2hd+dy, 2wd+dx]
      xk = x rows [64s-1, 64s+64) + mask channel, columns pre-deinterleaved
      (even/odd) on host so matmul rhs reads are step-1.
  kw = softmax_k(enc) in [k-partition, (hd, wd)-free] layout: exp on ACT,
      sum-over-k via ones-matmul on PE, reciprocal + normalize on DVE.
      kw -> DRAM scratch (piecewise, per 2-chunk group) -> partition-
      broadcast DMA back as kwb[(q,co), k, hh, w'] tiles (64-way partition
      replication), exports/broadcasts split across sync/scalar queues.
  G duplicated into both partition halves in one matmul (out_w tiled to
      [128, 128] lhsT; PE cost is column-bound so duplication is free):
      pg[(q,co), u] = sum_{c,t} out_w[co, 4c+t] x[c, row, u-2], evicted
      straight to a single shifted-read tile g2[(q,co), r, u'] =
      G[co, r, 128q + u' - 2] (132 cols) - q=0 cols on ACT, q=1 on DVE.
      All 25 tap shifts are then plain AP offset slices (no SWDGE copies).
  products: per half H, 25 taps k=(ki,kj):
      stg[(q,co), hh, w'] = g2[:, 8H+ki:+8, kj:kj+128] * kwb  (DVE 2x /
      Pool split), accumulated with identity matmuls into PSUM on PE;
      out_b added during ACT eviction (bias vector).
"""
import os

import numpy as np
import ml_dtypes

import concourse.bass as bass
import concourse.tile as tile
from concourse import bacc, mybir, masks
from concourse.bass_utils import run_bass_kernel_spmd

F32 = mybir.dt.float32
BF16 = mybir.dt.bfloat16

N_CORES = 8


# ----------------------------------------------------------------------------
# device program
# ----------------------------------------------------------------------------
def build_nc():
    nc = bacc.Bacc(None, target_bir_lowering=False)

    xk_d = nc.dram_tensor("xk", [2, 65, 69, 129], BF16, kind="ExternalInput")
    xb_d = nc.dram_tensor("xb", [2, 128, 20, 264], BF16, kind="ExternalInput")
    ct_d = nc.dram_tensor("ctap", [128, 5, 25], BF16, kind="ExternalInput")
    cm_d = nc.dram_tensor("ctm", [9, 25], BF16, kind="ExternalInput")
    xm_d = nc.dram_tensor("xm", [9, 32, 128], BF16, kind="ExternalInput")
    w4_d = nc.dram_tensor("w4", [2, 128, 128], BF16, kind="ExternalInput")
    ob_d = nc.dram_tensor("obv", [128, 1], F32, kind="ExternalInput")
    kwd_d = nc.dram_tensor("kwd", [2, 2, 8, 25, 128], BF16, kind="Internal")
    # out[H, (q,co), hh, w']; h' = 16H + 2hh + q
    out_d = nc.dram_tensor("out", [2, 128, 8, 128], BF16, kind="ExternalOutput")

    ctx = nc.allow_low_precision(reason="bf16 pipeline; validated ~1% rel err")
    ctx.__enter__()
    with tile.TileContext(nc) as tc:
        with (
            tc.tile_pool(name="consts", bufs=1) as consts,
            tc.tile_pool(name="xkp", bufs=3) as xkp,
            tc.tile_pool(name="xbp", bufs=1) as xbp,
            tc.tile_pool(name="g2p", bufs=1) as g2p,
            tc.tile_pool(name="kwp", bufs=1) as kwp,
            tc.tile_pool(name="encp", bufs=3) as encp,
            tc.tile_pool(name="kwbp", bufs=1) as kwbp,
            tc.tile_pool(name="stgp", bufs=2) as stgp,
            tc.tile_pool(name="resp", bufs=2) as resp,
            tc.tile_pool(name="pse", bufs=3, space="PSUM") as pse,
            tc.tile_pool(name="psg", bufs=2, space="PSUM") as psg,
            tc.tile_pool(name="pss", bufs=1, space="PSUM") as pss,
            tc.tile_pool(name="psacc", bufs=1, space="PSUM") as psacc,
        ):
            # ---- constants ----
            ctap = consts.tile([128, 5, 25], BF16)
            nc.sync.dma_start(ctap[:], ct_d[:])
            ctm = consts.tile([9, 25], BF16)
            nc.sync.dma_start(ctm[:], cm_d[:])
            xm = consts.tile([9, 32, 128], BF16)
            nc.scalar.dma_start(xm[:], xm_d[:])
            w4t = consts.tile([128, 2, 128], BF16)
            nc.scalar.dma_start(w4t[:], w4_d[:].transpose([1, 0, 2]))
            obv = consts.tile([128, 1], F32)
            nc.sync.dma_start(obv[:], ob_d[:])
            identb = consts.tile([128, 128], BF16)
            masks.make_identity(nc, identb[:])
            ones25 = consts.tile([25, 25], BF16)
            nc.gpsimd.memset(ones25[:], 1.0)

            # ---- input streams (xk on scalar queue; sync queue is the
            # kw export/broadcast pipeline) ----
            xkc = [[], []]

            def issue_xk(cp):
                # xkdup tiles [128, 18, 128] (one per 2-chunk pair):
                # partitions 0:64 = channels as-is; 64:128 = same channels
                # shifted (+1 col for par0 pairing dx=0/2, +2 rows for
                # par1 pairing dy=0/2).
                r0 = 16 * cp
                for par in range(2):
                    t = xkp.tile([128, 18, 128], BF16, tag=f"xk{par}",
                                 name=f"xk{par}_{cp}")
                    off = 1 if par == 0 else 2 * 129
                    eng = nc.scalar if par == 0 else nc.sync
                    e = xk_d[par, 0:64, r0:r0 + 18, 0:128]
                    eng.dma_start(t[0:64], e)
                    eng.dma_start(
                        t[64:128],
                        bass.AP(e.tensor, e.offset + off,
                                [list(x) for x in e.ap]))
                    xkc[par].append(t)

            issue_xk(0)
            issue_xk(1)
            xbt = xbp.tile([128, 2, 20, 264], BF16)
            nc.sync.dma_start(xbt[:], xb_d[:].transpose([1, 0, 2, 3]))
            issue_xk(2)

            kwn = kwp.tile([25, 32, 128], BF16)
            g2 = g2p.tile([128, 20, 132], BF16)
            kwec = {}

            def enc_chunk(cc):
                pe = pse.tile([25, 4, 128], F32, name=f"pe{cc}", tag="pe")
                cp, rb = cc // 2, 8 * (cc % 2)
                for dy in range(3):  # tap pairs (dy,0)+(dy,2)
                    nc.tensor.matmul(pe[:], ctap[:, dy, :],
                                     xkc[0][cp][:, rb + dy:rb + dy + 8:2, :],
                                     start=(dy == 0), stop=False)
                # pair (0,1)+(2,1), then center (1,1) (zero bottom rows)
                nc.tensor.matmul(pe[:], ctap[:, 3, :],
                                 xkc[1][cp][:, rb:rb + 8:2, :],
                                 start=False, stop=False)
                nc.tensor.matmul(pe[:], ctap[:, 4, :],
                                 xkc[1][cp][:, rb + 1:rb + 9:2, :],
                                 start=False, stop=False)
                # mask/bias taps (9-contraction)
                nc.tensor.matmul(pe[:], ctm[:],
                                 xm[:, 4 * cc:4 * cc + 4, :],
                                 start=False, stop=True)
                kwec[cc] = encp.tile([25, 4, 128], BF16, tag="kwe",
                                    name=f"kwe{cc}")
                nc.scalar.activation(kwec[cc][:], pe[:],
                                     mybir.ActivationFunctionType.Exp)

            def sum_chunk(cc):
                ps = pss.tile([25, 4, 128], F32, name=f"ps{cc}", tag="ps")
                nc.tensor.matmul(ps[:], ones25[:], kwec[cc][:],
                                 start=True, stop=True)
                rcp = encp.tile([25, 4, 128], F32, tag="rcp",
                               name=f"rcp{cc}")
                nc.vector.reciprocal_approx_fast(rcp[:], ps[:])
                nc.vector.tensor_mul(kwn[:, 4 * cc:4 * cc + 4, :],
                                     kwec[cc][:], rcp[:])

            # kw export + broadcast-back, piecewise per 2-chunk group.
            # half H, c2 in {0,1}: kwn rows hd in [16H+8c2, 16H+8c2+8)
            # = hh in [4c2, 4c2+4) for both q.  q=0 traffic on sync
            # queue, q=1 on scalar (same-queue RAW with its export).
            kwb = {}
            for H in range(2):
                for g in range(5):
                    kwb[(H, g)] = kwbp.tile([128, 8, 5, 128], BF16,
                                            name=f"kwb{H}{g}", tag=f"kwb{H}{g}")

            def kw_piece(H, c2):
                hd0 = 16 * H + 8 * c2
                for q in range(2):
                    nc.gpsimd.dma_start(
                        kwd_d[H, q, 4 * c2:4 * c2 + 4, :, :]
                        .transpose([1, 0, 2]),
                        kwn[:, hd0 + q:hd0 + 8:2, :])
                for g in range(5):
                    for q in range(2):
                        eng = nc.gpsimd if q == 1 else nc.sync
                        eng.dma_start(
                            kwb[(H, g)][64 * q:64 * q + 64,
                                        4 * c2:4 * c2 + 4, :, :],
                            kwd_d[H, q, 4 * c2:4 * c2 + 4, 5 * g:5 * g + 5]
                            .unsqueeze(0)
                            .broadcast_to([64, 4, 5, 128]))

            # ---- enc phase: all 8 chunks; sums pipelined one behind so
            # the PE never waits on the ACT exp eviction; kw pieces fire
            # as soon as their 2-chunk group is normalized ----
            enc_chunk(0)
            for cc in range(1, 8):
                if cc == 2:
                    issue_xk(3)
                enc_chunk(cc)
                sum_chunk(cc - 1)
                if cc % 2 == 0:
                    kw_piece((cc - 2) // 4, ((cc - 2) % 4) // 2)
            sum_chunk(7)
            kw_piece(1, 1)

            # ---- G rows: duplicated-G matmul, direct eviction to g2 ----
            for r in range(20):
                pg = psg.tile([128, 264], F32, tag="pg", name=f"pg{r}")
                for ci in range(2):
                    nc.tensor.matmul(pg[:], w4t[:, ci, :], xbt[:, ci, r, :],
                                     start=(ci == 0), stop=(ci == 1))
                nc.scalar.copy(g2[0:64, r, :], pg[0:64, 0:132])
                nc.vector.tensor_copy(g2[64:128, r, :], pg[64:128, 128:260])

            # ---- products ----
            def prod_half(H):
                acc = [psacc.tile([128, 512], F32, name=f"acc{H}{b}",
                                  tag=f"acc{b}") for b in range(2)]
                for ki in range(5):
                    stg = stgp.tile([128, 8, 5, 128], BF16, tag="stg",
                                    name=f"stg{H}_{ki}")
                    base = g2[:, 8 * H + ki:8 * H + ki + 8, 0:128]
                    # overlapped window AP: [p, hh(+132), kj(+1), w(+1)]
                    gsl = bass.AP(base.tensor, base.offset,
                                  [[base.ap[0][0], 128], [132, 8],
                                   [1, 5], [1, 128]])
                    nc.vector.tensor_mul(stg[:], gsl, kwb[(H, ki)][:])
                    for kj in range(5):
                        for b in range(2):
                            nc.tensor.matmul(
                                acc[b][:], identb[:],
                                stg[:, 4 * b:4 * b + 4, kj, :],
                                start=(ki == 0 and kj == 0),
                                stop=(ki == 4 and kj == 4))
                res = resp.tile([128, 8, 128], BF16, tag="res",
                                name=f"res{H}")
                for b in range(2):
                    nc.scalar.activation(
                        res[:, 4 * b:4 * b + 4, :], acc[b][:],
                        mybir.ActivationFunctionType.Identity,
                        bias=obv[:, 0].unsqueeze(-1))
                eng = nc.sync if H == 0 else nc.scalar
                eng.dma_start(out_d[H], res[:])

            prod_half(0)
            prod_half(1)

    nc.compile()
    ctx.__exit__(None, None, None)
    return nc


# ----------------------------------------------------------------------------
# host side
# ----------------------------------------------------------------------------
def _prep_weights(down_w, down_b, enc_w, enc_b, out_w, out_b):
    A = np.zeros((65, 65), np.float32)
    A[0:64, 0:64] = down_w[:, :, 0, 0]
    A[0:64, 64] = down_b
    A[64, 64] = 1.0
    ctap = np.zeros((65, 9, 25), np.float32)
    for dy in range(3):
        for dx in range(3):
            B = np.zeros((25, 65), np.float32)
            B[:, 0:64] = enc_w[:, :, dy, dx]
            if dy == 1 and dx == 1:
                B[:, 64] = enc_b
            ctap[:, 3 * dy + dx, :] = (B @ A).T
    body, ctm = ctap[0:64], ctap[64]          # [64, 9, 25], [9, 25]
    ctp = np.zeros((128, 5, 25), np.float32)
    for dy in range(3):                        # pairs (dy,0)+(dy,2)
        ctp[0:64, dy] = body[:, 3 * dy + 0]
        ctp[64:128, dy] = body[:, 3 * dy + 2]
    ctp[0:64, 3] = body[:, 1]                  # pair (0,1)+(2,1)
    ctp[64:128, 3] = body[:, 7]
    ctp[0:64, 4] = body[:, 4]                  # center (1,1); bottom 0
    w4 = out_w[:, :, 0, 0].T.reshape(2, 128, 64)
    w4 = np.tile(w4, (1, 1, 2)).astype(ml_dtypes.bfloat16)
    obv = np.tile(out_b, 2).reshape(128, 1).astype(np.float32)
    return (ctp.astype(ml_dtypes.bfloat16), ctm.astype(ml_dtypes.bfloat16),
            w4, obv)


def _slice_core(x, n, s):
    xk = np.zeros((65, 69, 258), np.float32)
    h0 = 64 * s - 1
    lo, hi = max(0, -h0), min(65, 256 - h0)
    xk[0:64, lo:hi, 1:257] = x[n, :, h0 + lo:h0 + hi, :]
    xkp = np.zeros((2, 65, 69, 129), np.float32)
    xkp[0] = xk[:, :, 0::2]
    xkp[1] = xk[:, :, 1::2]
    # mask tensor xm[t, hd, wd]: 1 where tap (dy,dx)=(t//3,t%3) lands on
    # a valid (unpadded) input position
    rv = np.zeros(69, bool)
    rv[lo:hi] = True
    cv = np.zeros(258, bool)
    cv[1:257] = True
    xm = np.zeros((9, 32, 128), np.float32)
    hd, wd = np.arange(32), np.arange(128)
    for t in range(9):
        dy, dx = t // 3, t % 3
        xm[t] = (rv[2 * hd + dy][:, None] & cv[2 * wd + dx][None, :])
    xb = np.zeros((2, 128, 20, 264), np.float32)
    xbv = xb.reshape(256, 20, 264)
    for t in range(4):
        g0 = 64 * t + 16 * s - 2
        lo, hi = max(0, -g0), min(20, 256 - g0)
        xbv[np.arange(64) * 4 + t, lo:hi, 2:258] = x[n, :, g0 + lo:g0 + hi, :]
    return (xkp.astype(ml_dtypes.bfloat16), xb.astype(ml_dtypes.bfloat16),
            xm.astype(ml_dtypes.bfloat16))


_NC_CACHE = None
LAST_EXEC_NS = None


def kernel(x, down_w, down_b, enc_w, enc_b, out_w, out_b):
    global _NC_CACHE, LAST_EXEC_NS
    x = np.asarray(x, np.float32)
    ctp, ctm, w4, obv = _prep_weights(
        np.asarray(down_w, np.float32), np.asarray(down_b, np.float32),
        np.asarray(enc_w, np.float32), np.asarray(enc_b, np.float32),
        np.asarray(out_w, np.float32), np.asarray(out_b, np.float32))
    in_maps = []
    for core in range(N_CORES):
        n, s = core // 4, core % 4
        xkp, xb, xm = _slice_core(x, n, s)
        in_maps.append({"xk": xkp, "xb": xb, "ctap": ctp, "ctm": ctm,
                        "xm": xm, "w4": w4, "obv": obv})
    if _NC_CACHE is None:
        _NC_CACHE = build_nc()
    kw = {}
    if os.environ.get("CARAFE_TRACE"):
        kw = dict(trace=True, tmpdir=os.environ.get("CARAFE_TRACE_DIR"))
    res = run_bass_kernel_spmd(_NC_CACHE, in_maps, list(range(N_CORES)), **kw)
    if res.exec_time_ns is not None:
        LAST_EXEC_NS = res.exec_time_ns
    out = np.zeros((2, 64, 128, 128), np.float32)
    for core in range(N_CORES):
        n, s = core // 4, core % 4
        o = res.results[core]["out"].astype(np.float32)  # (H, (q,co), hh, w')
        o = o.reshape(2, 2, 64, 8, 128)                  # (H, q, co, hh, w')
        # h' = 16H + 2hh + q
        o = o.transpose(2, 0, 3, 1, 4).reshape(64, 32, 128)
        out[n, :, 32 * s:32 * s + 32, :] = o
    return out


# revision 22
# speedup vs baseline: 1.0329x; 1.0164x over previous
"""CARAFE-downsampling Trainium2 kernel (8-core SPMD, full I/O contract).

Per core (core = 4n + s; batch n, output-row slab h' in [32s, 32s+32)):

  enc logits fused down+enc (9 taps, C_tap = B_tap @ A on host):
      enc[e, hd, wd] = sum_tap C_tap.T @ xk[:, 2hd+dy, 2wd+dx]
      xk = x rows [64s-1, 64s+64) + mask channel, columns pre-deinterleaved
      (even/odd) on host so matmul rhs reads are step-1.
  kw = softmax_k(enc) in [k-partition, (hd, wd)-free] layout: exp on ACT,
      sum-over-k via ones-matmul on PE, reciprocal + normalize on DVE.
      kw -> DRAM scratch (piecewise, per 2-chunk group) -> partition-
      broadcast DMA back as kwb[(q,co), k, hh, w'] tiles (64-way partition
      replication), exports/broadcasts split across sync/scalar queues.
  G duplicated into both partition halves in one matmul (out_w tiled to
      [128, 128] lhsT; PE cost is column-bound so duplication is free):
      pg[(q,co), u] = sum_{c,t} out_w[co, 4c+t] x[c, row, u-2], evicted
      straight to a single shifted-read tile g2[(q,co), r, u'] =
      G[co, r, 128q + u' - 2] (132 cols) - q=0 cols on ACT, q=1 on DVE.
      All 25 tap shifts are then plain AP offset slices (no SWDGE copies).
  products: per half H, 25 taps k=(ki,kj):
      stg[(q,co), hh, w'] = g2[:, 8H+ki:+8, kj:kj+128] * kwb  (DVE 2x /
      Pool split), accumulated with identity matmuls into PSUM on PE;
      out_b added during ACT eviction (bias vector).
"""
import os

import numpy as np
import ml_dtypes

import concourse.bass as bass
import concourse.tile as tile
from concourse import bacc, mybir, masks
from concourse.bass_utils import run_bass_kernel_spmd

F32 = mybir.dt.float32
BF16 = mybir.dt.bfloat16

N_CORES = 8


# ----------------------------------------------------------------------------
# device program
# ----------------------------------------------------------------------------
def build_nc():
    nc = bacc.Bacc(None, target_bir_lowering=False)

    xk_d = nc.dram_tensor("xk", [2, 65, 69, 129], BF16, kind="ExternalInput")
    xb_d = nc.dram_tensor("xb", [2, 128, 20, 264], BF16, kind="ExternalInput")
    ct_d = nc.dram_tensor("ctap", [128, 5, 25], BF16, kind="ExternalInput")
    cm_d = nc.dram_tensor("ctm", [9, 25], BF16, kind="ExternalInput")
    xm_d = nc.dram_tensor("xm", [9, 32, 128], BF16, kind="ExternalInput")
    w4_d = nc.dram_tensor("w4", [2, 128, 128], BF16, kind="ExternalInput")
    ob_d = nc.dram_tensor("obv", [128, 1], F32, kind="ExternalInput")
    kwd_d = nc.dram_tensor("kwd", [2, 2, 8, 25, 128], BF16, kind="Internal")
    # out[H, (q,co), hh, w']; h' = 16H + 2hh + q
    out_d = nc.dram_tensor("out", [2, 128, 8, 128], BF16, kind="ExternalOutput")

    ctx = nc.allow_low_precision(reason="bf16 pipeline; validated ~1% rel err")
    ctx.__enter__()
    with tile.TileContext(nc) as tc:
        with (
            tc.tile_pool(name="consts", bufs=1) as consts,
            tc.tile_pool(name="xkp", bufs=3) as xkp,
            tc.tile_pool(name="xbp", bufs=1) as xbp,
            tc.tile_pool(name="g2p", bufs=1) as g2p,
            tc.tile_pool(name="kwp", bufs=1) as kwp,
            tc.tile_pool(name="encp", bufs=3) as encp,
            tc.tile_pool(name="kwbp", bufs=1) as kwbp,
            tc.tile_pool(name="stgp", bufs=2) as stgp,
            tc.tile_pool(name="resp", bufs=2) as resp,
            tc.tile_pool(name="pse", bufs=3, space="PSUM") as pse,
            tc.tile_pool(name="psg", bufs=2, space="PSUM") as psg,
            tc.tile_pool(name="pss", bufs=1, space="PSUM") as pss,
            tc.tile_pool(name="psacc", bufs=1, space="PSUM") as psacc,
        ):
            # ---- constants ----
            ctap = consts.tile([128, 5, 25], BF16)
            nc.sync.dma_start(ctap[:], ct_d[:])
            ctm = consts.tile([9, 25], BF16)
            nc.sync.dma_start(ctm[:], cm_d[:])
            xm = consts.tile([9, 32, 128], BF16)
            nc.scalar.dma_start(xm[:], xm_d[:])
            w4t = consts.tile([128, 2, 128], BF16)
            nc.scalar.dma_start(w4t[:], w4_d[:].transpose([1, 0, 2]))
            obv = consts.tile([128, 1], F32)
            nc.sync.dma_start(obv[:], ob_d[:])
            identb = consts.tile([128, 128], BF16)
            masks.make_identity(nc, identb[:])
            ones25 = consts.tile([25, 25], BF16)
            nc.gpsimd.memset(ones25[:], 1.0)

            # ---- input streams (xk on scalar queue; sync queue is the
            # kw export/broadcast pipeline) ----
            xkc = [[], []]

            def issue_xk(cp):
                # xkdup tiles [128, 18, 128] (one per 2-chunk pair):
                # partitions 0:64 = channels as-is; 64:128 = same channels
                # shifted (+1 col for par0 pairing dx=0/2, +2 rows for
                # par1 pairing dy=0/2).
                r0 = 16 * cp
                for par in range(2):
                    t = xkp.tile([128, 18, 128], BF16, tag=f"xk{par}",
                                 name=f"xk{par}_{cp}")
                    off = 1 if par == 0 else 2 * 129
                    eng = nc.scalar if par == 0 else nc.sync
                    e = xk_d[par, 0:64, r0:r0 + 18, 0:128]
                    eng.dma_start(t[0:64], e)
                    eng.dma_start(
                        t[64:128],
                        bass.AP(e.tensor, e.offset + off,
                                [list(x) for x in e.ap]))
                    xkc[par].append(t)

            issue_xk(0)
            issue_xk(1)
            issue_xk(2)
            xbt = xbp.tile([128, 2, 20, 264], BF16)

            kwn = kwp.tile([25, 32, 128], BF16)
            g2 = g2p.tile([128, 20, 132], BF16)
            kwec = {}

            def enc_chunk(cc):
                pe = pse.tile([25, 4, 128], F32, name=f"pe{cc}", tag="pe")
                cp, rb = cc // 2, 8 * (cc % 2)
                for dy in range(3):  # tap pairs (dy,0)+(dy,2)
                    nc.tensor.matmul(pe[:], ctap[:, dy, :],
                                     xkc[0][cp][:, rb + dy:rb + dy + 8:2, :],
                                     start=(dy == 0), stop=False)
                # pair (0,1)+(2,1), then center (1,1) (zero bottom rows)
                nc.tensor.matmul(pe[:], ctap[:, 3, :],
                                 xkc[1][cp][:, rb:rb + 8:2, :],
                                 start=False, stop=False)
                nc.tensor.matmul(pe[:], ctap[:, 4, :],
                                 xkc[1][cp][:, rb + 1:rb + 9:2, :],
                                 start=False, stop=False)
                # mask/bias taps (9-contraction)
                nc.tensor.matmul(pe[:], ctm[:],
                                 xm[:, 4 * cc:4 * cc + 4, :],
                                 start=False, stop=True)
                kwec[cc] = encp.tile([25, 4, 128], BF16, tag="kwe",
                                    name=f"kwe{cc}")
                nc.scalar.activation(kwec[cc][:], pe[:],
                                     mybir.ActivationFunctionType.Exp)

            def sum_chunk(cc):
                ps = pss.tile([25, 4, 128], F32, name=f"ps{cc}", tag="ps")
                nc.tensor.matmul(ps[:], ones25[:], kwec[cc][:],
                                 start=True, stop=True)
                rcp = encp.tile([25, 4, 128], F32, tag="rcp",
                               name=f"rcp{cc}")
                nc.vector.reciprocal_approx_fast(rcp[:], ps[:])
                nc.vector.tensor_mul(kwn[:, 4 * cc:4 * cc + 4, :],
                                     kwec[cc][:], rcp[:])

            # kw export + broadcast-back, piecewise per 2-chunk group.
            # half H, c2 in {0,1}: kwn rows hd in [16H+8c2, 16H+8c2+8)
            # = hh in [4c2, 4c2+4) for both q.  q=0 traffic on sync
            # queue, q=1 on scalar (same-queue RAW with its export).
            kwb = {}
            for H in range(2):
                for g in range(5):
                    kwb[(H, g)] = kwbp.tile([128, 8, 5, 128], BF16,
                                            name=f"kwb{H}{g}", tag=f"kwb{H}{g}")

            def kw_piece(H, c2):
                hd0 = 16 * H + 8 * c2
                for q in range(2):
                    nc.gpsimd.dma_start(
                        kwd_d[H, q, 4 * c2:4 * c2 + 4, :, :]
                        .transpose([1, 0, 2]),
                        kwn[:, hd0 + q:hd0 + 8:2, :])
                for g in range(5):
                    for q in range(2):
                        eng = nc.gpsimd if q == 1 else nc.sync
                        eng.dma_start(
                            kwb[(H, g)][64 * q:64 * q + 64,
                                        4 * c2:4 * c2 + 4, :, :],
                            kwd_d[H, q, 4 * c2:4 * c2 + 4, 5 * g:5 * g + 5]
                            .unsqueeze(0)
                            .broadcast_to([64, 4, 5, 128]))

            # ---- enc phase: all 8 chunks; sums pipelined one behind so
            # the PE never waits on the ACT exp eviction; kw pieces fire
            # as soon as their 2-chunk group is normalized ----
            enc_chunk(0)
            for cc in range(1, 8):
                if cc == 2:
                    issue_xk(3)
                enc_chunk(cc)
                sum_chunk(cc - 1)
                if cc % 2 == 0:
                    kw_piece((cc - 2) // 4, ((cc - 2) % 4) // 2)
            sum_chunk(7)
            kw_piece(1, 1)
            nc.sync.dma_start(xbt[:], xb_d[:].transpose([1, 0, 2, 3]))

            # ---- G rows: duplicated-G matmul, direct eviction to g2 ----
            for r in range(20):
                pg = psg.tile([128, 264], F32, tag="pg", name=f"pg{r}")
                for ci in range(2):
                    nc.tensor.matmul(pg[:], w4t[:, ci, :], xbt[:, ci, r, :],
                                     start=(ci == 0), stop=(ci == 1))
                nc.scalar.copy(g2[0:64, r, :], pg[0:64, 0:132])
                nc.scalar.copy(g2[64:128, r, :], pg[64:128, 128:260])

            # ---- products ----
            def prod_half(H):
                acc = [psacc.tile([128, 512], F32, name=f"acc{H}{b}",
                                  tag=f"acc{b}") for b in range(2)]
                for ki in range(5):
                    stg = stgp.tile([128, 8, 5, 128], BF16, tag="stg",
                                    name=f"stg{H}_{ki}")
                    base = g2[:, 8 * H + ki:8 * H + ki + 8, 0:128]
                    # overlapped window AP: [p, hh(+132), kj(+1), w(+1)]
                    gsl = bass.AP(base.tensor, base.offset,
                                  [[base.ap[0][0], 128], [132, 8],
                                   [1, 5], [1, 128]])
                    nc.vector.tensor_mul(stg[:], gsl, kwb[(H, ki)][:])
                    for kj in range(5):
                        for b in range(2):
                            nc.tensor.matmul(
                                acc[b][:], identb[:],
                                stg[:, 4 * b:4 * b + 4, kj, :],
                                start=(ki == 0 and kj == 0),
                                stop=(ki == 4 and kj == 4))
                res = resp.tile([128, 8, 128], BF16, tag="res",
                                name=f"res{H}")
                for b in range(2):
                    nc.scalar.activation(
                        res[:, 4 * b:4 * b + 4, :], acc[b][:],
                        mybir.ActivationFunctionType.Identity,
                        bias=obv[:, 0].unsqueeze(-1))
                eng = nc.sync if H == 0 else nc.scalar
                eng.dma_start(out_d[H], res[:])

            prod_half(0)
            prod_half(1)

    nc.compile()
    ctx.__exit__(None, None, None)
    return nc


# ----------------------------------------------------------------------------
# host side
# ----------------------------------------------------------------------------
def _prep_weights(down_w, down_b, enc_w, enc_b, out_w, out_b):
    A = np.zeros((65, 65), np.float32)
    A[0:64, 0:64] = down_w[:, :, 0, 0]
    A[0:64, 64] = down_b
    A[64, 64] = 1.0
    ctap = np.zeros((65, 9, 25), np.float32)
    for dy in range(3):
        for dx in range(3):
            B = np.zeros((25, 65), np.float32)
            B[:, 0:64] = enc_w[:, :, dy, dx]
            if dy == 1 and dx == 1:
                B[:, 64] = enc_b
            ctap[:, 3 * dy + dx, :] = (B @ A).T
    body, ctm = ctap[0:64], ctap[64]          # [64, 9, 25], [9, 25]
    ctp = np.zeros((128, 5, 25), np.float32)
    for dy in range(3):                        # pairs (dy,0)+(dy,2)
        ctp[0:64, dy] = body[:, 3 * dy + 0]
        ctp[64:128, dy] = body[:, 3 * dy + 2]
    ctp[0:64, 3] = body[:, 1]                  # pair (0,1)+(2,1)
    ctp[64:128, 3] = body[:, 7]
    ctp[0:64, 4] = body[:, 4]                  # center (1,1); bottom 0
    w4 = out_w[:, :, 0, 0].T.reshape(2, 128, 64)
    w4 = np.tile(w4, (1, 1, 2)).astype(ml_dtypes.bfloat16)
    obv = np.tile(out_b, 2).reshape(128, 1).astype(np.float32)
    return (ctp.astype(ml_dtypes.bfloat16), ctm.astype(ml_dtypes.bfloat16),
            w4, obv)


def _slice_core(x, n, s):
    xk = np.zeros((65, 69, 258), np.float32)
    h0 = 64 * s - 1
    lo, hi = max(0, -h0), min(65, 256 - h0)
    xk[0:64, lo:hi, 1:257] = x[n, :, h0 + lo:h0 + hi, :]
    xkp = np.zeros((2, 65, 69, 129), np.float32)
    xkp[0] = xk[:, :, 0::2]
    xkp[1] = xk[:, :, 1::2]
    # mask tensor xm[t, hd, wd]: 1 where tap (dy,dx)=(t//3,t%3) lands on
    # a valid (unpadded) input position
    rv = np.zeros(69, bool)
    rv[lo:hi] = True
    cv = np.zeros(258, bool)
    cv[1:257] = True
    xm = np.zeros((9, 32, 128), np.float32)
    hd, wd = np.arange(32), np.arange(128)
    for t in range(9):
        dy, dx = t // 3, t % 3
        xm[t] = (rv[2 * hd + dy][:, None] & cv[2 * wd + dx][None, :])
    xb = np.zeros((2, 128, 20, 264), np.float32)
    xbv = xb.reshape(256, 20, 264)
    for t in range(4):
        g0 = 64 * t + 16 * s - 2
        lo, hi = max(0, -g0), min(20, 256 - g0)
        xbv[np.arange(64) * 4 + t, lo:hi, 2:258] = x[n, :, g0 + lo:g0 + hi, :]
    return (xkp.astype(ml_dtypes.bfloat16), xb.astype(ml_dtypes.bfloat16),
            xm.astype(ml_dtypes.bfloat16))


_NC_CACHE = None
LAST_EXEC_NS = None


def kernel(x, down_w, down_b, enc_w, enc_b, out_w, out_b):
    global _NC_CACHE, LAST_EXEC_NS
    x = np.asarray(x, np.float32)
    ctp, ctm, w4, obv = _prep_weights(
        np.asarray(down_w, np.float32), np.asarray(down_b, np.float32),
        np.asarray(enc_w, np.float32), np.asarray(enc_b, np.float32),
        np.asarray(out_w, np.float32), np.asarray(out_b, np.float32))
    in_maps = []
    for core in range(N_CORES):
        n, s = core // 4, core % 4
        xkp, xb, xm = _slice_core(x, n, s)
        in_maps.append({"xk": xkp, "xb": xb, "ctap": ctp, "ctm": ctm,
                        "xm": xm, "w4": w4, "obv": obv})
    if _NC_CACHE is None:
        _NC_CACHE = build_nc()
    kw = {}
    if os.environ.get("CARAFE_TRACE"):
        kw = dict(trace=True, tmpdir=os.environ.get("CARAFE_TRACE_DIR"))
    res = run_bass_kernel_spmd(_NC_CACHE, in_maps, list(range(N_CORES)), **kw)
    if res.exec_time_ns is not None:
        LAST_EXEC_NS = res.exec_time_ns
    out = np.zeros((2, 64, 128, 128), np.float32)
    for core in range(N_CORES):
        n, s = core // 4, core % 4
        o = res.results[core]["out"].astype(np.float32)  # (H, (q,co), hh, w')
        o = o.reshape(2, 2, 64, 8, 128)                  # (H, q, co, hh, w')
        # h' = 16H + 2hh + q
        o = o.transpose(2, 0, 3, 1, 4).reshape(64, 32, 128)
        out[n, :, 32 * s:32 * s + 32, :] = o
    return out
